# revision 1
# baseline (speedup 1.0000x reference)
"""Trainium2 Bass kernel for nn_KB_Mapping_19361712570541 (dense_cnn).

Math (W=1 image dim folded away; h = x.T in [C, N] channels-on-partition):
  dw3(h, w)[c,n] = w0[c]h[c,n-1] + w1[c]h[c,n] + w2[c]h[c,n+1]   (zero pad)
  b1  = relu(W1pw @ relu(dw3(h, wd1)))
  b2  = (relu(W21x1 @ h) + b1) * mask
  b2  = relu(W2pw @ relu(dw3(b2, wd2)))
  out = relu(Wf[:, :C] @ h + Wf[:, C:] @ b2)          -> out.T is [N, C]

Sharding: data-parallel along N across 8 cores; each core's input slab
carries halos of x/mask so no cross-core communication is needed.

Implementation notes (cost-model driven):
- All matmuls except the fusion's Wfh@h run as fp8e4 DoubleRow pairs
  (0.5 cycles/row, two K=128 planes per instruction). Depthwise taps
  pair {t0,t2} (plane stride +2 — odd plane strides are rejected by
  codegen) with {t1, zero-plane} as the second DR; single GEMMs ride a
  zero second weight plane at half cost. The accuracy-critical fusion
  term Wfh@h runs in fp16 (x shipped both as fp8 and fp16); measured
  end-to-end rel err ~6e-4.
- Six PSUM->SBUF evacuations per 510-col tile are the floor: DVE takes
  {relu d1, relu+add (STT), relu d2}, ACT {relu b1, relu b2, relu out};
  the mask multiply runs on Pool. One PSUM bank per tensor (d1 and the
  fusion accumulator double-buffered; 8 banks total).
- The emission is software-pipelined with skew: iteration i emits
  front_a(i) (dw1/d1s/b1p/b1r), back_dve(i-3) (dw2/d2s), front_b(i-1)
  (b2a/STT/mask), back_rest(i-3) (b2p/b2r/fusion), out_relu(i-4) — so
  per iteration the in-order queues see DVE:[d1s(i), d2s(i-3),
  STT(i-1)] and ACT:[b1r(i), b2r(i-3), o(i-4)] with every dependency
  (including the ~1.3us Pool mask latency) already satisfied. Two
  mid-stream tiles run d2s on ACT for balance; the last three tiles run
  b2r on DVE because ACT serializes the drain.
- DMAs are batched into graduated groups (sizes hill-climbed; each
  dma_start costs ~625ns of serial HWDGE time); the fp16 fusion slab
  rides one iteration later than the critical fp8 slab, and a narrow
  leading tile plus narrow trailing tiles shorten pipeline fill/drain.
  Work-tile ring depths (b2b 3, b1r 4, out 3) are tuned: extra slack
  there removes WAR hiccups, but deeper rings on d1s/d2s/b2r hurt.
- Zero-plane DRs read up to 2 columns past the producer's content; the
  fp8 work tiles are 516 wide and fully memset once per buffer (the
  interpreter hard-faults on uninitialized reads).
"""

import numpy as np
from contextlib import ExitStack

import ml_dtypes

import concourse.bass as bass
import concourse.bacc as bacc
import concourse.tile as tile
import concourse.mybir as mybir
from concourse.ap import AP
from concourse.bass_utils import run_bass_kernel_spmd

C = 128
N = 131072
NCORES = 8
NSH = N // NCORES          # 16384 output columns per core
T = 510                    # steady-state tile width (wE = 512 = one PSUM bank)
MASK_SEED = 42
MASK_P = 0.5

F32 = mybir.dt.float32
F16 = mybir.dt.float16
F8 = mybir.dt.float8e4
NP8 = ml_dtypes.float8_e4m3
DR = mybir.MatmulPerfMode.DoubleRow
Relu = mybir.ActivationFunctionType.Relu

# DR weight-pair indices in w8 (each pair is [C, 2, C] -> 256 cols)
P_D1A, P_D1B, P_W1, P_W21, P_D2A, P_D2B, P_W2, P_WFB = range(8)

LAST_RESULT = None         # BassKernelResults of the most recent run (for test.py)
TRACE = False

_mask_cache = None
_nc_cache = None


def _mask_cn() -> np.ndarray:
    """The reference's fixed Bernoulli mask in [C, N] layout, float32."""
    global _mask_cache
    if _mask_cache is None:
        import jax
        cpu = jax.devices("cpu")[0]
        with jax.default_device(cpu):
            m = jax.random.bernoulli(
                jax.random.key(MASK_SEED), 1.0 - MASK_P, (1, C, N, 1)
            )
            _mask_cache = np.asarray(m)[0, :, :, 0].astype(np.float32)
    return _mask_cache


def _tiles():
    """(a, width) list covering [0, NSH); narrow leader fills the pipe and
    narrow trailers drain it."""
    widths = [256] + [T] * 30 + [280, 282, 266]
    assert sum(widths) == NSH
    out, a = [], 0
    for w in widths:
        out.append((a, w))
        a += w
    return out


def _groups(tiles):
    """Graduated DMA groups as slices of the tile list."""
    sizes = [4, 5, 7, 8, 9]
    gs, i = [], 0
    for s in sizes:
        if i >= len(tiles):
            break
        gs.append(tiles[i:i + s])
        i += s
    if i < len(tiles):
        gs.append(tiles[i:])
    return gs


def _dr_rhs(t, col, n, delta=2):
    """[C, 2, n] moving AP over tile t: plane0 at col, plane1 at col+delta."""
    base = t[:, col:col + n]
    return AP(base.tensor, base.offset,
              [list(base.ap[0]), [delta, 2], [1, n]])


def _build_nc():
    nc = bacc.Bacc("TRN2", target_bir_lowering=False)

    x8 = nc.dram_tensor("x8", [C, NSH + 8], F8, kind="ExternalInput")
    x16 = nc.dram_tensor("x16", [C, NSH], F16, kind="ExternalInput")
    mk = nc.dram_tensor("mk", [C, NSH + 2], F8, kind="ExternalInput")
    w8 = nc.dram_tensor("w8", [C, 8 * 2 * C], F8, kind="ExternalInput")
    wf16 = nc.dram_tensor("wf16", [C, C], F16, kind="ExternalInput")
    y = nc.dram_tensor("y", [C, NSH], F16, kind="ExternalOutput")

    tiles = _tiles()
    groups = _groups(tiles)
    gw_max = max(sum(w for _, w in g) for g in groups)
    group_of = {}
    for gi, g in enumerate(groups):
        for t_ in g:
            group_of[t_[0]] = gi

    with ExitStack() as ctx:
        tc = ctx.enter_context(tile.TileContext(nc))
        wpool = ctx.enter_context(tc.tile_pool(name="weights", bufs=1))
        slab = ctx.enter_context(tc.tile_pool(name="slab", bufs=3))
        opool = ctx.enter_context(tc.tile_pool(name="out", bufs=3))
        work = ctx.enter_context(tc.tile_pool(name="work", bufs=3))
        ps = ctx.enter_context(tc.tile_pool(name="ps", bufs=1, space="PSUM"))

        w8_sb = wpool.tile([C, 8 * 2 * C], F8)
        wf_sb = wpool.tile([C, C], F16)

        def wpair(k):
            return w8_sb[:, k * 2 * C:(k + 1) * 2 * C].rearrange(
                "p (two m) -> p two m", two=2)

        # One-time zero of every buffer whose tail columns are read by
        # zero-plane DRs (delta-2 planes reach 2 cols past the written
        # content; the interpreter hard-faults on uninitialized reads).
        ZBUFS = {"d1s": 2, "b2m": 3, "d2s": 2, "b2r": 2}
        for ztag, zb in ZBUFS.items():
            for _ in range(zb):
                zt = work.tile([C, 516], F8, tag=ztag, bufs=zb,
                               name=f"z_{ztag}")
                nc.gpsimd.memset(zt[:, :], 0.0)

        # per-group slab state
        cur = {}

        pending_hs = []

        def load_group(gi):
            g = groups[gi]
            ga = g[0][0]
            gw = sum(w for _, w in g)
            xs = slab.tile([C, gw_max + 6], F8, tag="xs")
            nc.sync.dma_start(out=xs[:, :gw + 6], in_=x8[:, ga:ga + gw + 6])
            if gi == 0:
                # interleave the weight loads so dw1's pairs (first 768
                # cols) land right after the first x slab: the leading
                # tile's matmuls start ~2 HWDGE slots in.
                nc.sync.dma_start(out=w8_sb[:, :512], in_=w8[:, :512])
            ms = slab.tile([C, gw_max + 2], F8, tag="ms")
            nc.sync.dma_start(out=ms[:, :gw + 2], in_=mk[:, ga:ga + gw + 2])
            if gi == 0:
                nc.sync.dma_start(out=w8_sb[:, 512:], in_=w8[:, 512:])
            # the fp16 slab is only read by the fusion (lag 3): defer its
            # DMA so the next group's critical x slab gets the HWDGE first
            hs = slab.tile([C, gw_max], F16, tag="hs")
            o_c = opool.tile([C, gw_max], F16, tag="oc")
            cur[gi] = dict(ga=ga, gw=gw, xs=xs, hs=hs, ms=ms, o_c=o_c,
                           flushed=0, done=0)
            pending_hs.append(gi)

        def flush_hs():
            while pending_hs:
                gi = pending_hs.pop(0)
                g = cur[gi]
                nc.sync.dma_start(out=g["hs"][:, :g["gw"]],
                                  in_=x16[:, g["ga"]:g["ga"] + g["gw"]])
                if gi == 0:
                    nc.sync.dma_start(out=wf_sb[:, :], in_=wf16[:, :])

        def front_a(st):
            """dw1 -> d1s (DVE) -> b1p -> b1r (ACT)."""
            g = cur[st["gi"]]
            la, wE = st["la"], st["wE"]
            xs = g["xs"]
            d1p = ps.tile([C, 512], F32, tag="d1", bufs=2, name="d1p")
            nc.tensor.matmul(d1p[:, :wE], wpair(P_D1A),
                             _dr_rhs(xs, la, wE),
                             start=True, stop=False, perf_mode=DR)
            nc.tensor.matmul(d1p[:, :wE], wpair(P_D1B),
                             _dr_rhs(xs, la + 1, wE),
                             start=False, stop=True, perf_mode=DR)
            d1s = work.tile([C, 516], F8, tag="d1s", bufs=2)
            nc.vector.tensor_scalar_max(d1s[:, :wE], d1p[:, :wE], 0.0)
            b1p = ps.tile([C, 512], F32, tag="b1", name="b1p")
            nc.tensor.matmul(b1p[:, :wE], wpair(P_W1), _dr_rhs(d1s, 0, wE),
                             start=True, stop=True, perf_mode=DR)
            b1r = work.tile([C, 512], F16, tag="b1r", bufs=4)
            nc.scalar.activation(b1r[:, :wE], b1p[:, :wE], Relu)
            st.update(b1r=b1r)

        def back_dve(st):
            """dw2 -> d2s (DVE; ACT on a couple of tiles for balance)."""
            P_ = st["P_"]
            d2p = ps.tile([C, 512], F32, tag="d2", name="d2p")
            b2m = st["b2m"]
            nc.tensor.matmul(d2p[:, :P_], wpair(P_D2A), _dr_rhs(b2m, 0, P_),
                             start=True, stop=False, perf_mode=DR)
            nc.tensor.matmul(d2p[:, :P_], wpair(P_D2B), _dr_rhs(b2m, 1, P_),
                             start=False, stop=True, perf_mode=DR)
            d2s = work.tile([C, 516], F8, tag="d2s", bufs=2)
            if st.get("d2s_on_act"):
                nc.scalar.activation(d2s[:, :P_], d2p[:, :P_], Relu)
            else:
                nc.vector.tensor_scalar_max(d2s[:, :P_], d2p[:, :P_], 0.0)
            st.update(d2s=d2s)

        def front_b(st):
            """b2a -> STT (DVE) -> mask (Pool)."""
            g = cur[st["gi"]]
            la, wE = st["la"], st["wE"]
            b2ap = ps.tile([C, 512], F32, tag="b2a", name="b2ap")
            nc.tensor.matmul(b2ap[:, :wE], wpair(P_W21),
                             _dr_rhs(g["xs"], la + 1, wE),
                             start=True, stop=True, perf_mode=DR)
            b2b = work.tile([C, 512], F16, tag="b2b", bufs=3)
            nc.vector.scalar_tensor_tensor(
                b2b[:, :wE], b2ap[:, :wE], 0.0, st["b1r"][:, :wE],
                mybir.AluOpType.max, mybir.AluOpType.add)
            b2m = work.tile([C, 516], F8, tag="b2m", bufs=3)
            nc.gpsimd.tensor_mul(b2m[:, :wE], b2b[:, :wE],
                                 g["ms"][:, la:la + wE])
            st.update(b2m=b2m)

        def back_rest(st):
            """b2p -> b2r (ACT) -> fusion matmuls."""
            g = cur[st["gi"]]
            la, P_ = st["la"], st["P_"]
            b2p = ps.tile([C, 512], F32, tag="b2", name="b2p")
            nc.tensor.matmul(b2p[:, :P_], wpair(P_W2),
                             _dr_rhs(st["d2s"], 0, P_),
                             start=True, stop=True, perf_mode=DR)
            b2r = work.tile([C, 516], F8, tag="b2r", bufs=2)
            if st.get("tail"):
                nc.vector.tensor_scalar_max(b2r[:, :P_], b2p[:, :P_], 0.0)
            else:
                nc.scalar.activation(b2r[:, :P_], b2p[:, :P_], Relu)
            fp = ps.tile([C, 512], F32, tag="f", bufs=2, name="fp")
            nc.tensor.matmul(fp[:, :P_], wf_sb[:, :], g["hs"][:, la:la + P_],
                             start=True, stop=False)
            nc.tensor.matmul(fp[:, :P_], wpair(P_WFB), _dr_rhs(b2r, 0, P_),
                             start=False, stop=True, perf_mode=DR)
            st.update(fp=fp)

        def out_relu(st):
            """final relu (ACT) + output flush bookkeeping."""
            g = cur[st["gi"]]
            la, P_ = st["la"], st["P_"]
            if st.get("last"):
                nc.vector.tensor_scalar_max(g["o_c"][:, la:la + P_],
                                            st["fp"][:, :P_], 0.0)
            else:
                nc.scalar.activation(g["o_c"][:, la:la + P_], st["fp"][:, :P_],
                                     Relu)
            g["done"] += 1
            ntiles = len(groups[st["gi"]])
            # flush every 2 finished tiles
            if g["done"] % 2 == 0 or g["done"] == ntiles:
                lo, hi = g["flushed"], la + P_
                nc.sync.dma_start(out=y[:, g["ga"] + lo:g["ga"] + hi],
                                  in_=g["o_c"][:, lo:hi])
                g["flushed"] = hi

        # software-pipelined emission with skew: per iteration i the engine
        # queues get  DVE:[d1s(i), d2s(i-3), STT(i-1)]
        #             ACT:[b1r(i), b2r(i-3), o(i-4)]
        # so no engine waits on a cross-engine round trip: the Pool mask op
        # of tile i-3 has had two full iterations to finish before dw2/d2s.
        flat = [t_ for g in groups for t_ in g]
        n = len(flat)
        sts = []
        loaded = 0

        def ensure_loaded(upto):
            nonlocal loaded
            while loaded <= min(upto, len(groups) - 1):
                load_group(loaded)
                loaded += 1

        ensure_loaded(0)
        for i in range(n + 4):
            if i < n:
                a, P_ = flat[i]
                gi = group_of[a]
                ensure_loaded(gi + 1)
                st = dict(a=a, P_=P_, wE=P_ + 2, gi=gi,
                          la=a - cur[gi]["ga"],
                          d2s_on_act=(i in (12, 22)),
                          tail=(i >= n - 3), last=(i == n - 1))
                sts.append(st)
                front_a(st)
                flush_hs()
            if 0 <= i - 3 < n:
                back_dve(sts[i - 3])
            if 0 <= i - 1 < n:
                front_b(sts[i - 1])
            if 0 <= i - 3 < n:
                back_rest(sts[i - 3])
            if 0 <= i - 4 < n:
                out_relu(sts[i - 4])

    nc.compile()
    return nc


def kernel(x, w_b1_dw, w_b1_pw, w_b2_1x1, w_b2_dw, w_b2_pw, w_fusion):
    global LAST_RESULT, _nc_cache

    x = np.asarray(x, dtype=np.float32)
    h = np.ascontiguousarray(x.T)
    mask = _mask_cn()

    # host-side shard prep: [C, N] layouts, zero-padded halos
    x8_pad = np.zeros((C, N + 8), dtype=NP8)
    x8_pad[:, 2:N + 2] = h.astype(NP8)
    x16_pad = h.astype(np.float16)
    mk_pad = np.zeros((C, N + 2), dtype=NP8)
    mk_pad[:, 1:N + 1] = mask.astype(NP8)

    def taps(wdw):  # [C,1,3,3] -> per-channel taps along N
        return np.asarray(wdw)[:, 0, :, 1]  # [C, 3]

    t1 = taps(w_b1_dw)
    t2 = taps(w_b2_dw)

    def diag8(v):
        return np.diag(v.astype(np.float32)).astype(NP8)

    def lhsT8(w):  # [O, I] -> [I, O] fp8
        return np.ascontiguousarray(np.asarray(w, dtype=np.float32).T).astype(NP8)

    zero = np.zeros((C, C), dtype=NP8)
    pairs = [
        (diag8(t1[:, 0]), diag8(t1[:, 2])),
        (diag8(t1[:, 1]), zero),
        (lhsT8(np.asarray(w_b1_pw)[:, :, 0, 0]), zero),
        (lhsT8(np.asarray(w_b2_1x1)[:, :, 0, 0]), zero),
        (diag8(t2[:, 0]), diag8(t2[:, 2])),
        (diag8(t2[:, 1]), zero),
        (lhsT8(np.asarray(w_b2_pw)[:, :, 0, 0]), zero),
        (lhsT8(np.asarray(w_fusion)[:, C:, 0, 0]), zero),
    ]
    w8_host = np.empty((C, 8 * 2 * C), dtype=NP8)
    for k, (p0, p1) in enumerate(pairs):
        w8_host[:, (2 * k) * C:(2 * k + 1) * C] = p0
        w8_host[:, (2 * k + 1) * C:(2 * k + 2) * C] = p1
    wf_host = np.ascontiguousarray(
        np.asarray(w_fusion)[:, :C, 0, 0].astype(np.float32).T
    ).astype(np.float16)

    in_maps = []
    for i in range(NCORES):
        s = i * NSH
        in_maps.append({
            "x8": np.ascontiguousarray(x8_pad[:, s:s + NSH + 8]),
            "x16": np.ascontiguousarray(x16_pad[:, s:s + NSH]),
            "mk": np.ascontiguousarray(mk_pad[:, s:s + NSH + 2]),
            "w8": w8_host,
            "wf16": wf_host,
        })

    if _nc_cache is None:
        _nc_cache = _build_nc()

    res = run_bass_kernel_spmd(
        _nc_cache, in_maps, core_ids=list(range(NCORES)), trace=TRACE
    )
    LAST_RESULT = res

    out = np.empty((C, N), dtype=np.float32)
    for i in range(NCORES):
        out[:, i * NSH:(i + 1) * NSH] = res.results[i]["y"].astype(np.float32)
    return np.ascontiguousarray(out.T)

